# revision 35
# baseline (speedup 1.0000x reference)
"""GPT2 eager causal attention (B=2, S=2048, D=1024, H=16, HD=64) on 8 TRN2 NeuronCores.

Sharding (data + head/tensor parallel): core c -> (batch b = c//4, head-group
g = c%4), 4 heads per group.

Per-core pipeline:
  1. host feeds x[b] pre-transposed -> xT [d, s] strips land via plain DMA
  2. QT,KT = wq/wk^T @ xT -> [256, S] transposed layouts (head-dim on partitions)
     V = xT^T @ wv -> [S, 4x65] natural with a ones-column per head (memset once)
  3. per q-chunk, head-PAIR loop: two score MMs at base partitions 0/64 pack the
     PE via row tiling (K=64 each); exp on ScalarE batched over the pair
     [128, 2, W] with the 1/sqrt(64) scale folded in; causal diagonal blocks
     masked by one strided multiply; OT[65, q] += V^T @ exp(ST) per head -- the
     ones-column makes row 64 the softmax denominator; normalization via rank-1
     ones matmuls broadcasting the reciprocal row (PE part deferred into the
     next pair's first unit so the in-order PE queue never stalls on the DVE
     reciprocal chain); fused mul PSUM x SBUF -> OTsb bf16
  4. c_proj partial = OT^T-slices @ w_proj[group rows], bf16 partials
  5. bf16 ReduceScatter(add) per chunk over each 4-core group. Attention chunks
     are [512,512,512,256,256] so the final RS is a small 0.5MB piece whose
     predecessor overlaps the last chunk's attention -> short tail. bf16
     outputs, host converts/reassembles.

Emission interleaves QKV(next chunk) and c_proj(prev chunk) groups into the
attention stream so the PE stays dense (HAM stays at K=8/8) while ScalarE works
through the exps.

HW-validated quirks this code respects (CoreSim does not model them):
  - custom-DVE ops (reciprocal_*) must not remap partition offsets; copy to
    partition 0 first
  - a second col-tiled matmul into an already-written PSUM bank at
    tile_position (0,64) writes nothing; use one PSUM tile per matmul
  - engines cannot address SBUF at non-32-aligned partition bases
"""
from contextlib import ExitStack

import ml_dtypes
import numpy as np

import concourse.bacc as bacc
import concourse.mybir as mybir
import concourse.tile as tile
from concourse.bass_utils import run_bass_kernel_spmd

F32 = mybir.dt.float32
BF16 = mybir.dt.bfloat16

B, S, D, H, HD = 2, 2048, 1024, 16, 64
N_CORES = 8
HG = 4               # heads per group
DG = HG * HD         # 256 q/k channels per group
NK = D // 128        # 8 contraction tiles over d
NS = S // 128        # 16 token tiles
CH = 512             # max q-chunk width (one PSUM bank of fp32)
NCH = S // CH        # 4 QKV chunks
NRT = DG // 128      # 2 channel row-tiles per group
CHS = [512, 512, 512, 512]               # attention chunk widths
T0S = [0, 512, 1024, 1536]               # attention chunk token offsets
NCA = len(CHS)


def _build(has_bv: bool, has_bp: bool, has_bqk: bool = False, dbg: bool = False):
    nc = bacc.Bacc("TRN2", target_bir_lowering=False, debug=False, num_devices=N_CORES)

    x_d = nc.dram_tensor("x", [D, S], BF16, kind="ExternalInput").ap()
    wq_d = nc.dram_tensor("wq", [128, NK * DG], BF16, kind="ExternalInput").ap()
    wk_d = nc.dram_tensor("wk", [128, NK * DG], BF16, kind="ExternalInput").ap()
    wv_d = nc.dram_tensor("wv", [128, NK * DG], BF16, kind="ExternalInput").ap()
    wp_d = nc.dram_tensor("wp", [128, NRT * D], BF16, kind="ExternalInput").ap()
    bq_d = nc.dram_tensor("bq", [128, NRT], F32, kind="ExternalInput").ap()
    bk_d = nc.dram_tensor("bk", [128, NRT], F32, kind="ExternalInput").ap()
    bv_d = nc.dram_tensor("bv", [128, NRT], F32, kind="ExternalInput").ap()
    bp_d = nc.dram_tensor("bp", [128, D], F32, kind="ExternalInput").ap()
    mk_d = nc.dram_tensor("mk", [128, 128], BF16, kind="ExternalInput").ap()
    out_d = nc.dram_tensor("out", [CH, D], BF16, kind="ExternalOutput").ap()
    if dbg:
        dbg_qt = nc.dram_tensor("dbg_qt", [128, NRT, S], BF16, kind="ExternalOutput").ap()
        dbg_kt = nc.dram_tensor("dbg_kt", [128, NRT, S], BF16, kind="ExternalOutput").ap()
        dbg_v = nc.dram_tensor("dbg_v", [128, NS, HG, HD + 1], BF16, kind="ExternalOutput").ap()
        dbg_ot = nc.dram_tensor("dbg_ot", [128, NRT, S], BF16, kind="ExternalOutput").ap()
        dbg_den = nc.dram_tensor("dbg_den", [1, NCA * 2, 2, CH], F32, kind="ExternalOutput").ap()
        dbg_par = nc.dram_tensor("dbg_par", [S, D], BF16, kind="ExternalOutput").ap()

    EXP = mybir.ActivationFunctionType.Exp
    IDENT = mybir.ActivationFunctionType.Identity

    with ExitStack() as ctx:
        tc = ctx.enter_context(tile.TileContext(nc))
        persist = ctx.enter_context(tc.tile_pool(name="persist", bufs=1))
        stp = ctx.enter_context(tc.tile_pool(name="stp", bufs=3))
        rdp = ctx.enter_context(tc.tile_pool(name="rdp", bufs=2))
        # ob depth must cover a full chunk of c_proj casts: partial-write DMAs
        # stall while a ReduceScatter owns the shared DMA engines, and a full
        # ob pool would freeze the DVE FIFO (and with it the whole pipeline)
        obp = ctx.enter_context(tc.tile_pool(name="obp", bufs=9))
        ps_sc = ctx.enter_context(tc.tile_pool(name="ps_sc", bufs=2, space="PSUM"))
        ps_ot = ctx.enter_context(tc.tile_pool(name="ps_ot", bufs=2, space="PSUM"))
        ps_ms = ctx.enter_context(tc.tile_pool(name="ps_ms", bufs=2, space="PSUM"))
        dram = ctx.enter_context(tc.tile_pool(name="dram", bufs=1, space="DRAM"))

        # ---- persistent SBUF tiles
        wq_sb = persist.tile([128, NK * DG], BF16)
        wk_sb = persist.tile([128, NK * DG], BF16)
        wv_sb = persist.tile([128, NK * DG], BF16)
        wp_sb = persist.tile([128, NRT * D], BF16)
        mk_sb = persist.tile([128, 2, 128], BF16)
        on1 = persist.tile([1, 64], BF16)
        bq_sb = persist.tile([128, NRT], F32) if has_bqk else None
        bk_sb = persist.tile([128, NRT], F32) if has_bqk else None
        bv_sb = persist.tile([128, NRT], F32) if has_bv else None
        bp_sb = persist.tile([128, D], F32) if has_bp else None
        xT = [persist.tile([128, S], BF16, name=f"xT{d}") for d in range(NK)]
        QT = [persist.tile([128, S], BF16, name=f"qT{r}") for r in range(NRT)]
        KT = [persist.tile([128, S], BF16, name=f"kT{r}") for r in range(NRT)]
        OTsb = [persist.tile([128, S], BF16, name=f"oT{r}") for r in range(NRT)]
        V_all = persist.tile([128, NS, HG, HD + 1], BF16)

        # ---- input DMAs: wq then x chunk 0 first so warmup + QKV(0) start early
        nc.sync.dma_start(wq_sb[:], wq_d[:])
        for dt in range(NK):
            nc.sync.dma_start(xT[dt][:, 0:CH], x_d[dt * 128:(dt + 1) * 128, 0:CH])
        nc.sync.dma_start(wk_sb[:], wk_d[:])
        nc.sync.dma_start(wv_sb[:], wv_d[:])
        nc.sync.dma_start(wp_sb[:], wp_d[:])
        for j in range(2):
            nc.sync.dma_start(mk_sb[:, j, :], mk_d[:])
        if has_bqk:
            nc.sync.dma_start(bq_sb[:], bq_d[:])
            nc.sync.dma_start(bk_sb[:], bk_d[:])
        if has_bv:
            nc.sync.dma_start(bv_sb[:], bv_d[:])
        if has_bp:
            nc.sync.dma_start(bp_sb[:], bp_d[:])
        for ch in range(1, NCH):
            for dt in range(NK):
                nc.sync.dma_start(
                    xT[dt][:, ch * CH:(ch + 1) * CH],
                    x_d[dt * 128:(dt + 1) * 128, ch * CH:(ch + 1) * CH],
                )

        nc.vector.memset(on1[:], 1.0)
        nc.vector.memset(V_all[:, :, :, HD:HD + 1], 1.0)

        # ---- PE warmup: keep the array busy through the HAM window while x lands
        for i in range(12):
            wps = ps_ms.tile([128, CH], F32, tag="mm", name=f"warm{i}")
            nc.tensor.matmul(wps[:], wq_sb[:, 0:128], wq_sb[:, 0:CH], start=True, stop=True)

        # ---- QKV + c_proj group emitters (each rotates one misc PSUM bank)
        def emit_qkt_group(dst, w_sb, b_sb, rt, ch):
            ps = ps_ms.tile([128, CH], F32, tag="mm", name=f"qk{rt}_{ch}")
            for kt in range(NK):
                nc.tensor.matmul(
                    ps[:],
                    w_sb[:, kt * DG + rt * 128: kt * DG + (rt + 1) * 128],
                    xT[kt][:, ch * CH:(ch + 1) * CH],
                    start=(kt == 0), stop=(kt == NK - 1),
                )
            if has_bqk:
                nc.scalar.activation(
                    dst[:, ch * CH:(ch + 1) * CH], ps[:], IDENT,
                    bias=b_sb[:, rt:rt + 1],
                )
            else:
                nc.vector.tensor_copy(dst[:, ch * CH:(ch + 1) * CH], ps[:])

        def emit_v_group(st):
            ps = ps_ms.tile([128, HG, HD], F32, tag="mm", name=f"v{st}")
            for kt in range(NK):
                nc.tensor.matmul(
                    ps[:, :, :],
                    xT[kt][:, st * 128:(st + 1) * 128],
                    wv_sb[:, kt * DG:(kt + 1) * DG],
                    start=(kt == 0), stop=(kt == NK - 1),
                )
            nc.vector.tensor_copy(V_all[:, st, :, 0:HD], ps[:, :, :])

        partials = [
            dram.tile([CHS[c], D], BF16, tag=f"partial{c}", name=f"partial{c}")
            for c in range(NCA)
        ]

        def emit_cproj_group(ci, stl, n):
            tok = T0S[ci] + stl * 128
            ps = ps_ms.tile([128, CH], F32, tag="mm", name=f"po{ci}_{stl}_{n}")
            for k2 in range(NRT):
                nc.tensor.matmul(
                    ps[:],
                    OTsb[k2][:, tok:tok + 128],
                    wp_sb[:, k2 * D + n * CH: k2 * D + (n + 1) * CH],
                    start=(k2 == 0), stop=(k2 == NRT - 1),
                )
            ob = obp.tile([128, CH], BF16, tag="ob", name=f"ob{ci}_{stl}_{n}")
            if has_bp:
                nc.vector.tensor_add(ob[:], ps[:], bp_sb[:, n * CH:(n + 1) * CH])
            else:
                nc.vector.tensor_copy(ob[:], ps[:])
            nc.sync.dma_start(
                partials[ci][stl * 128:(stl + 1) * 128, n * CH:(n + 1) * CH], ob[:]
            )

        def emit_rs(ci, half):
            rows = 64
            rs_c = dram.tile([rows, D], BF16, tag=f"rs{ci}_{half}", name=f"rs_{ci}_{half}")
            nc.gpsimd.collective_compute(
                "ReduceScatter",
                mybir.AluOpType.add,
                replica_groups=[[0, 1, 2, 3], [4, 5, 6, 7]],
                ins=[partials[ci][256 * half:256 * (half + 1), :].opt()],
                outs=[rs_c.opt()],
            )
            o = T0S[ci] // 4 + 64 * half
            # out DMA rides the gpsimd queue: it depends on the RS this queue
            # already blocks on, so it can never stall compute dispatch (the
            # scheduler would hoist a scalar/sync-queue DMA ahead of the exps)
            nc.gpsimd.dma_start(out_d[o:o + rows, :], rs_c[:])

        # ---- attention emitters (scores and OT split so OT trails one kt
        # behind: the PE queue always has the next scores before an exp-gated
        # OT, keeping the array streaming while ScalarE works)
        def attn_sc(ci, p, kt):
            t0k = T0S[ci] // 128
            W = CHS[ci]
            d = kt - t0k  # >=0 on the causal diagonal strip
            sc = ps_sc.tile([128, 2, CH], F32, tag="sc", name=f"sc{ci}_{p}_{kt}")
            for j in range(2):
                off = 64 * j
                nc.tensor.matmul(
                    sc[:, j, 0:W],
                    KT[p][off:off + 64, kt * 128:(kt + 1) * 128],
                    QT[p][off:off + 64, T0S[ci]:T0S[ci] + W],
                    start=True, stop=True,
                )
            st = stp.tile([128, 2, CH], BF16, tag="st", name=f"st{ci}_{p}_{kt}")
            a = max(d, 0) * 128
            nc.scalar.activation(st[:, :, a:W], sc[:, :, a:W], EXP, scale=0.125)
            if d >= 0:
                nc.vector.tensor_mul(
                    st[:, :, d * 128:(d + 1) * 128],
                    st[:, :, d * 128:(d + 1) * 128],
                    mk_sb[:, :, :],
                )
            return st

        def attn_ot(ci, p, kt, nkt, ot_pair, st):
            t0k = T0S[ci] // 128
            W = CHS[ci]
            a = max(kt - t0k, 0) * 128
            for j in range(2):
                hl = 2 * p + j
                nc.tensor.matmul(
                    ot_pair[j][:, a:W],
                    V_all[:, kt, hl, :],
                    st[:, j, a:W],
                    start=(kt == 0), stop=(kt == nkt - 1),
                )

        def pair_norm_pre(ci, p, ot_pair):
            """DVE-only reciprocal chain; returns the deferred PE+mul closure."""
            W = CHS[ci]
            den = rdp.tile([1, 2, CH], F32, tag="den", name=f"den{ci}_{p}")
            nc.vector.tensor_copy(den[:, 0, 0:W], ot_pair[0][64:65, 0:W])
            nc.vector.tensor_copy(den[:, 1, 0:W], ot_pair[1][64:65, 0:W])
            rdf = rdp.tile([1, 2, CH], F32, tag="rdf", name=f"rdf{ci}_{p}")
            nc.vector.reciprocal_approx_fast(rdf[:, :, 0:W], den[:, :, 0:W])
            if dbg:
                nc.sync.dma_start(dbg_den[:, 2 * ci + p, :, :], den[:])
            rdb = rdp.tile([1, 2, CH], BF16, tag="rdb", name=f"rdb{ci}_{p}")
            nc.vector.tensor_copy(rdb[:, :, 0:W], rdf[:, :, 0:W])

            def post():
                for j in range(2):
                    # rank-1 broadcast of the reciprocal row to 64 partitions;
                    # one PSUM tile per matmul (HW quirk, see module docstring)
                    rbp = ps_ms.tile([64, CH], F32, tag="mm", name=f"rb{ci}_{p}_{j}")
                    nc.tensor.matmul(rbp[:, 0:W], on1[:], rdb[:, j, 0:W], start=True, stop=True)
                    rbs = rdp.tile([64, CH], BF16, tag=f"rbs{j}", name=f"rbs{ci}_{p}_{j}")
                    nc.vector.tensor_copy(rbs[:, 0:W], rbp[:, 0:W])
                    dst = OTsb[p][64 * j:64 * j + 64, T0S[ci]:T0S[ci] + W]
                    nc.vector.tensor_mul(dst, ot_pair[j][0:64, 0:W], rbs[:, 0:W])
                    if has_bv:
                        nc.vector.tensor_scalar_add(
                            dst, dst, bv_sb[64 * j:64 * j + 64, p:p + 1]
                        )
            return post

        def qt_groups(ch):
            return [
                (lambda rt=rt, ch=ch: emit_qkt_group(QT[rt], wq_sb, bq_sb, rt, ch))
                for rt in range(NRT)
            ]

        def ktv_groups(ch):
            gs = [
                (lambda rt=rt, ch=ch: emit_qkt_group(KT[rt], wk_sb, bk_sb, rt, ch))
                for rt in range(NRT)
            ]
            for st4 in range(4):
                gs.append(lambda st=4 * ch + st4: emit_v_group(st))
            return gs

        def cproj_half(ci, half):
            gs = []
            for stl in (2 * half, 2 * half + 1):
                for n in range(NRT):
                    gs.append(lambda ci=ci, stl=stl, n=n: emit_cproj_group(ci, stl, n))
            gs.append(lambda ci=ci, half=half: emit_rs(ci, half))
            return gs

        # ---- prologue: QKV chunks 0-2 (dense warm PE block; ScalarE has
        # nothing to do yet anyway). Chunk 3's QKV slides into attn(2).
        prologue = []
        for c in range(3):
            prologue += qt_groups(c)
            prologue += ktv_groups(c)
        for g in prologue:
            g()

        # ---- attention chunks processed big-first (2,3,1,0): the heavy exp
        # chunks run early with real filler work (chunk 3's QKV, then c_proj),
        # each RS hides under a later chunk's attention, and only the last
        # chunk's RS is exposed as tail. KT/V of QKV-chunk c are only read by
        # attention units with kt >= 4c, so qkv(3) legally fills attn(2).
        # First c_proj half of each chunk fires immediately at the chunk's end
        # (starting its RS as early as possible); the second half is filler for
        # the next chunk.
        order = [2, 3, 1, 0]
        fill_map = {
            2: (qt_groups(3) + ktv_groups(3), None),
            3: (cproj_half(2, 1), None),
            1: (cproj_half(3, 1), None),
            0: (cproj_half(1, 1), None),
        }

        # pending_post carries the previous pair's normalization PE work into
        # the head of the next pair's PE stream (after its first scores).
        pending_post = None
        for ci in order:
            W = CHS[ci]
            nkt = (T0S[ci] + W) // 128
            fillers, deadline = fill_map[ci]
            n_units = 2 * nkt
            span = deadline if deadline is not None else n_units
            fi = 0
            ui = 0
            for p in range(2):
                ot_pair = [
                    ps_ot.tile([65, CH], F32, tag="ot", name=f"ot{ci}_{p}_{j}")
                    for j in range(2)
                ]
                st_prev = None
                for kt in range(nkt):
                    st = attn_sc(ci, p, kt)
                    if kt == 0 and pending_post is not None:
                        pending_post()
                        pending_post = None
                    if kt > 0:
                        attn_ot(ci, p, kt - 1, nkt, ot_pair, st_prev)
                    st_prev = st
                    ui += 1
                    want = min((ui * len(fillers)) // span, len(fillers))
                    while fi < want:
                        fillers[fi]()
                        fi += 1
                attn_ot(ci, p, nkt - 1, nkt, ot_pair, st_prev)
                pending_post = pair_norm_pre(ci, p, ot_pair)
                # give the PE real work to chew while the DVE reciprocal chain
                # runs, so the deferred rank-1 broadcast MMs don't stall it
                for _ in range(2):
                    if fi < len(fillers):
                        fillers[fi]()
                        fi += 1
            while fi < len(fillers):
                fillers[fi]()
                fi += 1
            # flush the last pair's norm, then kick off this chunk's first
            # c_proj half so its ReduceScatter enters the stream right away
            pending_post()
            pending_post = None
            for g in cproj_half(ci, 0):
                g()

        # ---- epilogue: second c_proj half of the last-processed chunk
        for g in cproj_half(order[-1], 1):
            g()

        if dbg:
            for rt in range(NRT):
                nc.sync.dma_start(dbg_qt[:, rt, :], QT[rt][:])
                nc.sync.dma_start(dbg_kt[:, rt, :], KT[rt][:])
                nc.sync.dma_start(dbg_ot[:, rt, :], OTsb[rt][:])
            nc.sync.dma_start(dbg_v[:], V_all[:])
            for ci in range(NCA):
                nc.sync.dma_start(dbg_par[T0S[ci]:T0S[ci] + CHS[ci], :], partials[ci][:, :])

    nc.compile()
    return nc


_prog_cache = {}


def _get_prog(has_bv, has_bp, has_bqk):
    key = (has_bv, has_bp, has_bqk)
    if key not in _prog_cache:
        _prog_cache[key] = _build(has_bv, has_bp, has_bqk)
    return _prog_cache[key]


def _prepare(x, w_attn, b_attn, w_proj, b_proj):
    x = np.asarray(x, dtype=np.float32)
    w_attn = np.asarray(w_attn, dtype=np.float32)
    b_attn = np.asarray(b_attn, dtype=np.float32)
    w_proj = np.asarray(w_proj, dtype=np.float32)
    b_proj = np.asarray(b_proj, dtype=np.float32)

    has_bv = bool(np.any(b_attn[2 * D:]))
    has_bp = bool(np.any(b_proj))
    has_bqk = bool(np.any(b_attn[:2 * D]))
    nc = _get_prog(has_bv, has_bp, has_bqk)

    ii = np.arange(128)[:, None]
    jj = np.arange(128)[None, :]
    mask = (jj >= ii).astype(np.float32).astype(ml_dtypes.bfloat16)

    def tile_cols(w, c0, width):
        t = np.empty((128, NK * width), np.float32)
        for kt in range(NK):
            t[:, kt * width:(kt + 1) * width] = w[kt * 128:(kt + 1) * 128, c0:c0 + width]
        return t.astype(ml_dtypes.bfloat16)

    xb = [np.ascontiguousarray(x[b].T).astype(ml_dtypes.bfloat16) for b in range(B)]

    per_group = []
    for g in range(HG):
        q0 = g * DG
        k0 = D + g * DG
        v0 = 2 * D + g * DG
        wp_t = np.empty((128, NRT * D), np.float32)
        for rt in range(NRT):
            wp_t[:, rt * D:(rt + 1) * D] = w_proj[g * DG + rt * 128: g * DG + (rt + 1) * 128, :]
        bt = {}
        for nm, c0 in (("bq", q0), ("bk", k0), ("bv", v0)):
            t = np.empty((128, NRT), np.float32)
            for rt in range(NRT):
                t[:, rt] = b_attn[c0 + rt * 128: c0 + (rt + 1) * 128]
            bt[nm] = t
        if g == 0:
            bp_tile = np.broadcast_to(b_proj, (128, D)).astype(np.float32)
        else:
            bp_tile = np.zeros((128, D), dtype=np.float32)
        per_group.append({
            "wq": tile_cols(w_attn, q0, DG),
            "wk": tile_cols(w_attn, k0, DG),
            "wv": tile_cols(w_attn, v0, DG),
            "wp": wp_t.astype(ml_dtypes.bfloat16),
            "bq": np.ascontiguousarray(bt["bq"]),
            "bk": np.ascontiguousarray(bt["bk"]),
            "bv": np.ascontiguousarray(bt["bv"]),
            "bp": np.ascontiguousarray(bp_tile),
            "mk": mask,
        })

    in_maps = []
    for c in range(N_CORES):
        b, g = divmod(c, 4)
        m = dict(per_group[g])
        m["x"] = xb[b]
        in_maps.append(m)
    return nc, in_maps


def _assemble(results):
    out = np.empty((B, S, D), dtype=np.float32)
    for c in range(N_CORES):
        b, g = divmod(c, 4)
        o = np.asarray(results[c]["out"], dtype=np.float32)
        for ci in range(NCA):
            for half in range(2):
                r0 = T0S[ci] // 4 + 64 * half
                tok = T0S[ci] + 256 * half + g * 64
                out[b, tok:tok + 64, :] = o[r0:r0 + 64, :]
    return out


def kernel(x, w_attn, b_attn, w_proj, b_proj):
    nc, in_maps = _prepare(x, w_attn, b_attn, w_proj, b_proj)
    res = run_bass_kernel_spmd(nc, in_maps, list(range(N_CORES)))
    return _assemble(res.results)


# revision 40
# speedup vs baseline: 1.1306x; 1.1306x over previous
"""GPT2 eager causal attention (B=2, S=2048, D=1024, H=16, HD=64) on 8 TRN2 NeuronCores.

Sharding (data + head/tensor parallel): core c -> (batch b = c//4, head-group
g = c%4), 4 heads per group.

Per-core pipeline:
  1. host feeds x[b] pre-transposed -> xT [d, s] strips land via plain DMA
  2. QT,KT = wq/wk^T @ xT -> [256, S] transposed layouts (head-dim on partitions)
     V = xT^T @ wv -> [S, 4x65] natural with a ones-column per head (memset once)
  3. per q-chunk, head-PAIR loop: two score MMs at base partitions 0/64 pack the
     PE via row tiling (K=64 each); exp on ScalarE batched over the pair
     [128, 2, W] with the 1/sqrt(64) scale folded in; causal diagonal blocks
     masked by one strided multiply; OT[65, q] += V^T @ exp(ST) per head -- the
     ones-column makes row 64 the softmax denominator; normalization via rank-1
     ones matmuls broadcasting the reciprocal row (PE part deferred into the
     next pair's first unit so the in-order PE queue never stalls on the DVE
     reciprocal chain); fused mul PSUM x SBUF -> OTsb bf16
  4. c_proj partial = OT^T-slices @ w_proj[group rows], bf16 partials
  5. bf16 ReduceScatter(add) per chunk over each 4-core group. Attention chunks
     are [512,512,512,256,256] so the final RS is a small 0.5MB piece whose
     predecessor overlaps the last chunk's attention -> short tail. bf16
     outputs, host converts/reassembles.

Emission interleaves QKV(next chunk) and c_proj(prev chunk) groups into the
attention stream so the PE stays dense (HAM stays at K=8/8) while ScalarE works
through the exps.

HW-validated quirks this code respects (CoreSim does not model them):
  - custom-DVE ops (reciprocal_*) must not remap partition offsets; copy to
    partition 0 first
  - a second col-tiled matmul into an already-written PSUM bank at
    tile_position (0,64) writes nothing; use one PSUM tile per matmul
  - engines cannot address SBUF at non-32-aligned partition bases
"""
from contextlib import ExitStack

import ml_dtypes
import numpy as np

import concourse.bacc as bacc
import concourse.mybir as mybir
import concourse.tile as tile
from concourse.bass_utils import run_bass_kernel_spmd

F32 = mybir.dt.float32
BF16 = mybir.dt.bfloat16

B, S, D, H, HD = 2, 2048, 1024, 16, 64
N_CORES = 8
HG = 4               # heads per group
DG = HG * HD         # 256 q/k channels per group
NK = D // 128        # 8 contraction tiles over d
NS = S // 128        # 16 token tiles
CH = 512             # max q-chunk width (one PSUM bank of fp32)
NCH = S // CH        # 4 QKV chunks
NRT = DG // 128      # 2 channel row-tiles per group
CHS = [512, 512, 512, 512]               # attention chunk widths
T0S = [0, 512, 1024, 1536]               # attention chunk token offsets
NCA = len(CHS)


def _build(has_bv: bool, has_bp: bool, has_bqk: bool = False, dbg: bool = False):
    nc = bacc.Bacc("TRN2", target_bir_lowering=False, debug=False, num_devices=N_CORES)

    x_d = nc.dram_tensor("x", [D, S], BF16, kind="ExternalInput").ap()
    wq_d = nc.dram_tensor("wq", [128, NK * DG], BF16, kind="ExternalInput").ap()
    wk_d = nc.dram_tensor("wk", [128, NK * DG], BF16, kind="ExternalInput").ap()
    wv_d = nc.dram_tensor("wv", [128, NK * DG], BF16, kind="ExternalInput").ap()
    wp_d = nc.dram_tensor("wp", [128, NRT * D], BF16, kind="ExternalInput").ap()
    bq_d = nc.dram_tensor("bq", [128, NRT], F32, kind="ExternalInput").ap()
    bk_d = nc.dram_tensor("bk", [128, NRT], F32, kind="ExternalInput").ap()
    bv_d = nc.dram_tensor("bv", [128, NRT], F32, kind="ExternalInput").ap()
    bp_d = nc.dram_tensor("bp", [128, D], F32, kind="ExternalInput").ap()
    mk_d = nc.dram_tensor("mk", [128, 128], BF16, kind="ExternalInput").ap()
    out_d = nc.dram_tensor("out", [CH, D], BF16, kind="ExternalOutput").ap()
    if dbg:
        dbg_qt = nc.dram_tensor("dbg_qt", [128, NRT, S], BF16, kind="ExternalOutput").ap()
        dbg_kt = nc.dram_tensor("dbg_kt", [128, NRT, S], BF16, kind="ExternalOutput").ap()
        dbg_v = nc.dram_tensor("dbg_v", [128, NS, HG, HD + 1], BF16, kind="ExternalOutput").ap()
        dbg_ot = nc.dram_tensor("dbg_ot", [128, NRT, S], BF16, kind="ExternalOutput").ap()
        dbg_den = nc.dram_tensor("dbg_den", [1, NCA * 2, 2, CH], F32, kind="ExternalOutput").ap()
        dbg_par = nc.dram_tensor("dbg_par", [S, D], BF16, kind="ExternalOutput").ap()

    EXP = mybir.ActivationFunctionType.Exp
    IDENT = mybir.ActivationFunctionType.Identity

    with ExitStack() as ctx:
        tc = ctx.enter_context(tile.TileContext(nc))
        persist = ctx.enter_context(tc.tile_pool(name="persist", bufs=1))
        stp = ctx.enter_context(tc.tile_pool(name="stp", bufs=3))
        rdp = ctx.enter_context(tc.tile_pool(name="rdp", bufs=2))
        # ob depth must cover a full chunk of c_proj casts: partial-write DMAs
        # stall while a ReduceScatter owns the shared DMA engines, and a full
        # ob pool would freeze the DVE FIFO (and with it the whole pipeline)
        obp = ctx.enter_context(tc.tile_pool(name="obp", bufs=9))
        ps_sc = ctx.enter_context(tc.tile_pool(name="ps_sc", bufs=2, space="PSUM"))
        ps_ot = ctx.enter_context(tc.tile_pool(name="ps_ot", bufs=2, space="PSUM"))
        ps_ms = ctx.enter_context(tc.tile_pool(name="ps_ms", bufs=2, space="PSUM"))
        dram = ctx.enter_context(tc.tile_pool(name="dram", bufs=1, space="DRAM"))

        # ---- persistent SBUF tiles
        wq_sb = persist.tile([128, NK * DG], BF16)
        wk_sb = persist.tile([128, NK * DG], BF16)
        wv_sb = persist.tile([128, NK * DG], BF16)
        wp_sb = persist.tile([128, NRT * D], BF16)
        mk_sb = persist.tile([128, 2, 128], BF16)
        on1 = persist.tile([1, 64], BF16)
        bq_sb = persist.tile([128, NRT], F32) if has_bqk else None
        bk_sb = persist.tile([128, NRT], F32) if has_bqk else None
        bv_sb = persist.tile([128, NRT], F32) if has_bv else None
        bp_sb = persist.tile([128, D], F32) if has_bp else None
        xT = [persist.tile([128, S], BF16, name=f"xT{d}") for d in range(NK)]
        QT = [persist.tile([128, S], BF16, name=f"qT{r}") for r in range(NRT)]
        KT = [persist.tile([128, S], BF16, name=f"kT{r}") for r in range(NRT)]
        OTsb = [persist.tile([128, S], BF16, name=f"oT{r}") for r in range(NRT)]
        V_all = persist.tile([128, NS, HG, HD + 1], BF16)

        # ---- input DMAs: wq then x chunk 0 first so warmup + QKV(0) start early
        nc.sync.dma_start(wq_sb[:], wq_d[:])
        for dt in range(NK):
            nc.sync.dma_start(xT[dt][:, 0:CH], x_d[dt * 128:(dt + 1) * 128, 0:CH])
        nc.sync.dma_start(wk_sb[:], wk_d[:])
        nc.sync.dma_start(wv_sb[:], wv_d[:])
        nc.sync.dma_start(wp_sb[:], wp_d[:])
        for j in range(2):
            nc.sync.dma_start(mk_sb[:, j, :], mk_d[:])
        if has_bqk:
            nc.sync.dma_start(bq_sb[:], bq_d[:])
            nc.sync.dma_start(bk_sb[:], bk_d[:])
        if has_bv:
            nc.sync.dma_start(bv_sb[:], bv_d[:])
        if has_bp:
            nc.sync.dma_start(bp_sb[:], bp_d[:])
        for ch in range(1, NCH):
            for dt in range(NK):
                nc.sync.dma_start(
                    xT[dt][:, ch * CH:(ch + 1) * CH],
                    x_d[dt * 128:(dt + 1) * 128, ch * CH:(ch + 1) * CH],
                )

        nc.vector.memset(on1[:], 1.0)
        nc.vector.memset(V_all[:, :, :, HD:HD + 1], 1.0)

        # ---- PE warmup: keep the array busy through the HAM window while x lands
        for i in range(12):
            wps = ps_ms.tile([128, CH], F32, tag="mm", name=f"warm{i}")
            nc.tensor.matmul(wps[:], wq_sb[:, 0:128], wq_sb[:, 0:CH], start=True, stop=True)

        # ---- QKV + c_proj group emitters (each rotates one misc PSUM bank)
        def emit_qkt_group(dst, w_sb, b_sb, rt, ch):
            ps = ps_ms.tile([128, CH], F32, tag="mm", name=f"qk{rt}_{ch}")
            for kt in range(NK):
                nc.tensor.matmul(
                    ps[:],
                    w_sb[:, kt * DG + rt * 128: kt * DG + (rt + 1) * 128],
                    xT[kt][:, ch * CH:(ch + 1) * CH],
                    start=(kt == 0), stop=(kt == NK - 1),
                )
            if has_bqk:
                nc.scalar.activation(
                    dst[:, ch * CH:(ch + 1) * CH], ps[:], IDENT,
                    bias=b_sb[:, rt:rt + 1],
                )
            else:
                nc.vector.tensor_copy(dst[:, ch * CH:(ch + 1) * CH], ps[:])

        def emit_v_group(st):
            ps = ps_ms.tile([128, HG, HD], F32, tag="mm", name=f"v{st}")
            for kt in range(NK):
                nc.tensor.matmul(
                    ps[:, :, :],
                    xT[kt][:, st * 128:(st + 1) * 128],
                    wv_sb[:, kt * DG:(kt + 1) * DG],
                    start=(kt == 0), stop=(kt == NK - 1),
                )
            nc.vector.tensor_copy(V_all[:, st, :, 0:HD], ps[:, :, :])

        partials = [
            dram.tile([CHS[c], D], BF16, tag=f"partial{c}", name=f"partial{c}")
            for c in range(NCA)
        ]

        def emit_cproj_group(ci, stl, n):
            tok = T0S[ci] + stl * 128
            ps = ps_ms.tile([128, CH], F32, tag="mm", name=f"po{ci}_{stl}_{n}")
            for k2 in range(NRT):
                nc.tensor.matmul(
                    ps[:],
                    OTsb[k2][:, tok:tok + 128],
                    wp_sb[:, k2 * D + n * CH: k2 * D + (n + 1) * CH],
                    start=(k2 == 0), stop=(k2 == NRT - 1),
                )
            ob = obp.tile([128, CH], BF16, tag="ob", name=f"ob{ci}_{stl}_{n}")
            if has_bp:
                nc.vector.tensor_add(ob[:], ps[:], bp_sb[:, n * CH:(n + 1) * CH])
            else:
                nc.vector.tensor_copy(ob[:], ps[:])
            nc.sync.dma_start(
                partials[ci][stl * 128:(stl + 1) * 128, n * CH:(n + 1) * CH], ob[:]
            )

        def emit_rs(ci, half):
            # half in (0, 1) scatters 256 token rows; half=None the whole 512
            if half is None:
                rows, r0, o = 128, 0, T0S[ci] // 4
            else:
                rows, r0, o = 64, 256 * half, T0S[ci] // 4 + 64 * half
            rs_c = dram.tile([rows, D], BF16, tag=f"rs{ci}_{half}", name=f"rs_{ci}_{half}")
            nc.gpsimd.collective_compute(
                "ReduceScatter",
                mybir.AluOpType.add,
                replica_groups=[[0, 1, 2, 3], [4, 5, 6, 7]],
                ins=[partials[ci][r0:r0 + 4 * rows, :].opt()],
                outs=[rs_c.opt()],
            )
            # out DMA rides the gpsimd queue: it depends on the RS this queue
            # already blocks on, so it can never stall compute dispatch (the
            # scheduler would hoist a scalar/sync-queue DMA ahead of the exps)
            nc.gpsimd.dma_start(out_d[o:o + rows, :], rs_c[:])

        # ---- attention emitters (scores and OT split so OT trails one kt
        # behind: the PE queue always has the next scores before an exp-gated
        # OT, keeping the array streaming while ScalarE works)
        def attn_sc(ci, p, kt):
            t0k = T0S[ci] // 128
            W = CHS[ci]
            d = kt - t0k  # >=0 on the causal diagonal strip
            sc = ps_sc.tile([128, 2, CH], F32, tag="sc", name=f"sc{ci}_{p}_{kt}")
            for j in range(2):
                off = 64 * j
                nc.tensor.matmul(
                    sc[:, j, 0:W],
                    KT[p][off:off + 64, kt * 128:(kt + 1) * 128],
                    QT[p][off:off + 64, T0S[ci]:T0S[ci] + W],
                    start=True, stop=True,
                )
            st = stp.tile([128, 2, CH], BF16, tag="st", name=f"st{ci}_{p}_{kt}")
            a = max(d, 0) * 128
            nc.scalar.activation(st[:, :, a:W], sc[:, :, a:W], EXP, scale=0.125)
            if d >= 0:
                nc.vector.tensor_mul(
                    st[:, :, d * 128:(d + 1) * 128],
                    st[:, :, d * 128:(d + 1) * 128],
                    mk_sb[:, :, :],
                )
            return st

        def attn_ot(ci, p, kt, nkt, ot_pair, st):
            t0k = T0S[ci] // 128
            W = CHS[ci]
            a = max(kt - t0k, 0) * 128
            for j in range(2):
                hl = 2 * p + j
                nc.tensor.matmul(
                    ot_pair[j][:, a:W],
                    V_all[:, kt, hl, :],
                    st[:, j, a:W],
                    start=(kt == 0), stop=(kt == nkt - 1),
                )

        def pair_norm_pre(ci, p, ot_pair):
            """DVE-only reciprocal chain; returns the deferred PE+mul closure."""
            W = CHS[ci]
            den = rdp.tile([1, 2, CH], F32, tag="den", name=f"den{ci}_{p}")
            nc.vector.tensor_copy(den[:, 0, 0:W], ot_pair[0][64:65, 0:W])
            nc.vector.tensor_copy(den[:, 1, 0:W], ot_pair[1][64:65, 0:W])
            rdf = rdp.tile([1, 2, CH], F32, tag="rdf", name=f"rdf{ci}_{p}")
            nc.vector.reciprocal_approx_fast(rdf[:, :, 0:W], den[:, :, 0:W])
            if dbg:
                nc.sync.dma_start(dbg_den[:, 2 * ci + p, :, :], den[:])
            rdb = rdp.tile([1, 2, CH], BF16, tag="rdb", name=f"rdb{ci}_{p}")
            nc.vector.tensor_copy(rdb[:, :, 0:W], rdf[:, :, 0:W])

            def post():
                for j in range(2):
                    # rank-1 broadcast of the reciprocal row to 64 partitions;
                    # one PSUM tile per matmul (HW quirk, see module docstring)
                    rbp = ps_ms.tile([64, CH], F32, tag="mm", name=f"rb{ci}_{p}_{j}")
                    nc.tensor.matmul(rbp[:, 0:W], on1[:], rdb[:, j, 0:W], start=True, stop=True)
                    rbs = rdp.tile([64, CH], BF16, tag=f"rbs{j}", name=f"rbs{ci}_{p}_{j}")
                    nc.vector.tensor_copy(rbs[:, 0:W], rbp[:, 0:W])
                    dst = OTsb[p][64 * j:64 * j + 64, T0S[ci]:T0S[ci] + W]
                    nc.vector.tensor_mul(dst, ot_pair[j][0:64, 0:W], rbs[:, 0:W])
                    if has_bv:
                        nc.vector.tensor_scalar_add(
                            dst, dst, bv_sb[64 * j:64 * j + 64, p:p + 1]
                        )
            return post

        def qt_groups(ch):
            return [
                (lambda rt=rt, ch=ch: emit_qkt_group(QT[rt], wq_sb, bq_sb, rt, ch))
                for rt in range(NRT)
            ]

        def ktv_groups(ch):
            gs = [
                (lambda rt=rt, ch=ch: emit_qkt_group(KT[rt], wk_sb, bk_sb, rt, ch))
                for rt in range(NRT)
            ]
            for st4 in range(4):
                gs.append(lambda st=4 * ch + st4: emit_v_group(st))
            return gs

        def cproj_part(ci, half):
            stls = range(4) if half is None else (2 * half, 2 * half + 1)
            gs = []
            for stl in stls:
                for n in range(NRT):
                    gs.append(lambda ci=ci, stl=stl, n=n: emit_cproj_group(ci, stl, n))
            gs.append(lambda ci=ci, half=half: emit_rs(ci, half))
            return gs

        # ---- schedule. Chunks run smallest-first (0,2,3,1) so the first RS
        # enters the collective stream right after the startup barrier; each
        # chunk's c_proj runs immediately (fully or half) so its RS triggers
        # early, with remaining halves used as PE filler inside later chunks'
        # exp-bound attention. KT/V of QKV-chunk c are only read by attention
        # units with kt >= 4c, so they slide into the chunk itself under a
        # unit deadline.
        pending_post = [None]

        def attn_chunk(ci, fill_seqs):
            W = CHS[ci]
            nkt = (T0S[ci] + W) // 128
            n_units = 2 * nkt
            seqs = [(list(gs), span if span is not None else n_units)
                    for gs, span in fill_seqs]
            ui = 0
            # flatten: assign each filler an emission deadline unit
            flat = []
            u0 = 0
            for gs, span in seqs:
                for k, g in enumerate(gs):
                    flat.append((g, u0 + ((k + 1) * span) // len(gs)))
                u0 += span
            fi = 0
            for p in range(2):
                ot_pair = [
                    ps_ot.tile([65, CH], F32, tag="ot", name=f"ot{ci}_{p}_{j}")
                    for j in range(2)
                ]
                st_prev = None
                for kt in range(nkt):
                    st = attn_sc(ci, p, kt)
                    if kt == 0 and pending_post[0] is not None:
                        pending_post[0]()
                        pending_post[0] = None
                    if kt > 0:
                        attn_ot(ci, p, kt - 1, nkt, ot_pair, st_prev)
                    st_prev = st
                    ui += 1
                    while fi < len(flat) and flat[fi][1] <= ui:
                        flat[fi][0]()
                        fi += 1
                attn_ot(ci, p, nkt - 1, nkt, ot_pair, st_prev)
                pending_post[0] = pair_norm_pre(ci, p, ot_pair)
                # give the PE real work while the DVE reciprocal chain runs,
                # so the deferred rank-1 broadcast MMs don't stall it
                for _ in range(2):
                    if fi < len(flat):
                        flat[fi][0]()
                        fi += 1
            while fi < len(flat):
                flat[fi][0]()
                fi += 1
            # flush the last pair's norm so immediate c_proj can read OTsb
            pending_post[0]()
            pending_post[0] = None

        def block(gs):
            for g in gs:
                g()

        block(qt_groups(0) + ktv_groups(0) + qt_groups(1))
        attn_chunk(0, [(ktv_groups(1), None)])
        block(cproj_part(0, None) + qt_groups(2))          # RS(0) full, early
        attn_chunk(2, [(ktv_groups(2), 7), (qt_groups(3), None)])
        block(cproj_part(2, 0))                            # RS(2) first half
        attn_chunk(3, [(ktv_groups(3), 11), (cproj_part(2, 1), None)])
        block(cproj_part(3, 0))                            # RS(3) first half
        attn_chunk(1, [(cproj_part(3, 1), None)])
        block(cproj_part(1, None))                         # RS(1) full = tail

        if dbg:
            for rt in range(NRT):
                nc.sync.dma_start(dbg_qt[:, rt, :], QT[rt][:])
                nc.sync.dma_start(dbg_kt[:, rt, :], KT[rt][:])
                nc.sync.dma_start(dbg_ot[:, rt, :], OTsb[rt][:])
            nc.sync.dma_start(dbg_v[:], V_all[:])
            for ci in range(NCA):
                nc.sync.dma_start(dbg_par[T0S[ci]:T0S[ci] + CHS[ci], :], partials[ci][:, :])

    nc.compile()
    return nc


_prog_cache = {}


def _get_prog(has_bv, has_bp, has_bqk):
    key = (has_bv, has_bp, has_bqk)
    if key not in _prog_cache:
        _prog_cache[key] = _build(has_bv, has_bp, has_bqk)
    return _prog_cache[key]


def _prepare(x, w_attn, b_attn, w_proj, b_proj):
    x = np.asarray(x, dtype=np.float32)
    w_attn = np.asarray(w_attn, dtype=np.float32)
    b_attn = np.asarray(b_attn, dtype=np.float32)
    w_proj = np.asarray(w_proj, dtype=np.float32)
    b_proj = np.asarray(b_proj, dtype=np.float32)

    has_bv = bool(np.any(b_attn[2 * D:]))
    has_bp = bool(np.any(b_proj))
    has_bqk = bool(np.any(b_attn[:2 * D]))
    nc = _get_prog(has_bv, has_bp, has_bqk)

    ii = np.arange(128)[:, None]
    jj = np.arange(128)[None, :]
    mask = (jj >= ii).astype(np.float32).astype(ml_dtypes.bfloat16)

    def tile_cols(w, c0, width):
        t = np.empty((128, NK * width), np.float32)
        for kt in range(NK):
            t[:, kt * width:(kt + 1) * width] = w[kt * 128:(kt + 1) * 128, c0:c0 + width]
        return t.astype(ml_dtypes.bfloat16)

    xb = [np.ascontiguousarray(x[b].T).astype(ml_dtypes.bfloat16) for b in range(B)]

    per_group = []
    for g in range(HG):
        q0 = g * DG
        k0 = D + g * DG
        v0 = 2 * D + g * DG
        wp_t = np.empty((128, NRT * D), np.float32)
        for rt in range(NRT):
            wp_t[:, rt * D:(rt + 1) * D] = w_proj[g * DG + rt * 128: g * DG + (rt + 1) * 128, :]
        bt = {}
        for nm, c0 in (("bq", q0), ("bk", k0), ("bv", v0)):
            t = np.empty((128, NRT), np.float32)
            for rt in range(NRT):
                t[:, rt] = b_attn[c0 + rt * 128: c0 + (rt + 1) * 128]
            bt[nm] = t
        if g == 0:
            bp_tile = np.broadcast_to(b_proj, (128, D)).astype(np.float32)
        else:
            bp_tile = np.zeros((128, D), dtype=np.float32)
        per_group.append({
            "wq": tile_cols(w_attn, q0, DG),
            "wk": tile_cols(w_attn, k0, DG),
            "wv": tile_cols(w_attn, v0, DG),
            "wp": wp_t.astype(ml_dtypes.bfloat16),
            "bq": np.ascontiguousarray(bt["bq"]),
            "bk": np.ascontiguousarray(bt["bk"]),
            "bv": np.ascontiguousarray(bt["bv"]),
            "bp": np.ascontiguousarray(bp_tile),
            "mk": mask,
        })

    in_maps = []
    for c in range(N_CORES):
        b, g = divmod(c, 4)
        m = dict(per_group[g])
        m["x"] = xb[b]
        in_maps.append(m)
    return nc, in_maps


def _assemble(results):
    # chunks 0 and 1 reduce-scatter whole (rank gets 128 rows); 2 and 3 in two
    # 256-token halves (rank gets 64 rows per half)
    out = np.empty((B, S, D), dtype=np.float32)
    for c in range(N_CORES):
        b, g = divmod(c, 4)
        o = np.asarray(results[c]["out"], dtype=np.float32)
        for ci in (0, 1):
            r0 = T0S[ci] // 4
            tok = T0S[ci] + g * 128
            out[b, tok:tok + 128, :] = o[r0:r0 + 128, :]
        for ci in (2, 3):
            for half in range(2):
                r0 = T0S[ci] // 4 + 64 * half
                tok = T0S[ci] + 256 * half + g * 64
                out[b, tok:tok + 64, :] = o[r0:r0 + 64, :]
    return out


def kernel(x, w_attn, b_attn, w_proj, b_proj):
    nc, in_maps = _prepare(x, w_attn, b_attn, w_proj, b_proj)
    res = run_bass_kernel_spmd(nc, in_maps, list(range(N_CORES)))
    return _assemble(res.results)


# revision 44
# speedup vs baseline: 1.1698x; 1.0346x over previous
"""GPT2 eager causal attention (B=2, S=2048, D=1024, H=16, HD=64) on 8 TRN2 NeuronCores.

Sharding (data + head/tensor parallel): core c -> (batch b = c//4, head-group
g = c%4), 4 heads per group.

Per-core pipeline:
  1. host feeds x[b] pre-transposed -> xT [d, s] strips land via plain DMA
  2. QT,KT = wq/wk^T @ xT -> [256, S] transposed layouts (head-dim on partitions)
     V = xT^T @ wv -> [S, 4x65] natural with a ones-column per head (memset once)
  3. per q-chunk, head-PAIR loop: two score MMs at base partitions 0/64 pack the
     PE via row tiling (K=64 each); exp on ScalarE batched over the pair
     [128, 2, W] with the 1/sqrt(64) scale folded in; causal diagonal blocks
     masked by one strided multiply; OT[65, q] += V^T @ exp(ST) per head -- the
     ones-column makes row 64 the softmax denominator; normalization via rank-1
     ones matmuls broadcasting the reciprocal row (PE part deferred into the
     next pair's first unit so the in-order PE queue never stalls on the DVE
     reciprocal chain); fused mul PSUM x SBUF -> OTsb bf16
  4. c_proj partial = OT^T-slices @ w_proj[group rows], bf16 partials
  5. bf16 ReduceScatter(add) per chunk over each 4-core group. Attention chunks
     are [512,512,512,256,256] so the final RS is a small 0.5MB piece whose
     predecessor overlaps the last chunk's attention -> short tail. bf16
     outputs, host converts/reassembles.

Emission interleaves QKV(next chunk) and c_proj(prev chunk) groups into the
attention stream so the PE stays dense (HAM stays at K=8/8) while ScalarE works
through the exps.

HW-validated quirks this code respects (CoreSim does not model them):
  - custom-DVE ops (reciprocal_*) must not remap partition offsets; copy to
    partition 0 first
  - a second col-tiled matmul into an already-written PSUM bank at
    tile_position (0,64) writes nothing; use one PSUM tile per matmul
  - engines cannot address SBUF at non-32-aligned partition bases
"""
from contextlib import ExitStack

import ml_dtypes
import numpy as np

import concourse.bacc as bacc
import concourse.mybir as mybir
import concourse.tile as tile
from concourse.bass_utils import run_bass_kernel_spmd

F32 = mybir.dt.float32
BF16 = mybir.dt.bfloat16

B, S, D, H, HD = 2, 2048, 1024, 16, 64
N_CORES = 8
HG = 4               # heads per group
DG = HG * HD         # 256 q/k channels per group
NK = D // 128        # 8 contraction tiles over d
NS = S // 128        # 16 token tiles
CH = 512             # max q-chunk width (one PSUM bank of fp32)
NCH = S // CH        # 4 QKV chunks
NRT = DG // 128      # 2 channel row-tiles per group
CHS = [512, 512, 512, 512]               # attention chunk widths
T0S = [0, 512, 1024, 1536]               # attention chunk token offsets
NCA = len(CHS)


def _build(has_bv: bool, has_bp: bool, has_bqk: bool = False, dbg: bool = False):
    nc = bacc.Bacc("TRN2", target_bir_lowering=False, debug=False, num_devices=N_CORES)

    x_d = nc.dram_tensor("x", [D, S], BF16, kind="ExternalInput").ap()
    wq_d = nc.dram_tensor("wq", [128, NK * DG], BF16, kind="ExternalInput").ap()
    wk_d = nc.dram_tensor("wk", [128, NK * DG], BF16, kind="ExternalInput").ap()
    wv_d = nc.dram_tensor("wv", [128, NK * DG], BF16, kind="ExternalInput").ap()
    wp_d = nc.dram_tensor("wp", [128, NRT * D], BF16, kind="ExternalInput").ap()
    bq_d = nc.dram_tensor("bq", [128, NRT], F32, kind="ExternalInput").ap()
    bk_d = nc.dram_tensor("bk", [128, NRT], F32, kind="ExternalInput").ap()
    bv_d = nc.dram_tensor("bv", [128, NRT], F32, kind="ExternalInput").ap()
    bp_d = nc.dram_tensor("bp", [128, D], F32, kind="ExternalInput").ap()
    mk_d = nc.dram_tensor("mk", [128, 128], BF16, kind="ExternalInput").ap()
    out_d = nc.dram_tensor("out", [CH, D], BF16, kind="ExternalOutput").ap()
    if dbg:
        dbg_qt = nc.dram_tensor("dbg_qt", [128, NRT, S], BF16, kind="ExternalOutput").ap()
        dbg_kt = nc.dram_tensor("dbg_kt", [128, NRT, S], BF16, kind="ExternalOutput").ap()
        dbg_v = nc.dram_tensor("dbg_v", [128, NS, HG, HD + 1], BF16, kind="ExternalOutput").ap()
        dbg_ot = nc.dram_tensor("dbg_ot", [128, NRT, S], BF16, kind="ExternalOutput").ap()
        dbg_den = nc.dram_tensor("dbg_den", [1, NCA * 2, 2, CH], F32, kind="ExternalOutput").ap()
        dbg_par = nc.dram_tensor("dbg_par", [S, D], BF16, kind="ExternalOutput").ap()

    EXP = mybir.ActivationFunctionType.Exp
    IDENT = mybir.ActivationFunctionType.Identity

    with ExitStack() as ctx:
        tc = ctx.enter_context(tile.TileContext(nc))
        persist = ctx.enter_context(tc.tile_pool(name="persist", bufs=1))
        stp = ctx.enter_context(tc.tile_pool(name="stp", bufs=3))
        rdp = ctx.enter_context(tc.tile_pool(name="rdp", bufs=2))
        # ob depth must cover a full chunk of c_proj casts: partial-write DMAs
        # stall while a ReduceScatter owns the shared DMA engines, and a full
        # ob pool would freeze the DVE FIFO (and with it the whole pipeline)
        obp = ctx.enter_context(tc.tile_pool(name="obp", bufs=9))
        ps_sc = ctx.enter_context(tc.tile_pool(name="ps_sc", bufs=2, space="PSUM"))
        ps_ot = ctx.enter_context(tc.tile_pool(name="ps_ot", bufs=2, space="PSUM"))
        ps_ms = ctx.enter_context(tc.tile_pool(name="ps_ms", bufs=2, space="PSUM"))
        dram = ctx.enter_context(tc.tile_pool(name="dram", bufs=1, space="DRAM"))

        # ---- persistent SBUF tiles
        wq_sb = persist.tile([128, NK * DG], BF16)
        wk_sb = persist.tile([128, NK * DG], BF16)
        wv_sb = persist.tile([128, NK * DG], BF16)
        wp_sb = persist.tile([128, NRT * D], BF16)
        mk_sb = persist.tile([128, 2, 128], BF16)
        on1 = persist.tile([1, 64], BF16)
        bq_sb = persist.tile([128, NRT], F32) if has_bqk else None
        bk_sb = persist.tile([128, NRT], F32) if has_bqk else None
        bv_sb = persist.tile([128, NRT], F32) if has_bv else None
        bp_sb = persist.tile([128, D], F32) if has_bp else None
        xT = [persist.tile([128, S], BF16, name=f"xT{d}") for d in range(NK)]
        QT = [persist.tile([128, S], BF16, name=f"qT{r}") for r in range(NRT)]
        KT = [persist.tile([128, S], BF16, name=f"kT{r}") for r in range(NRT)]
        OTsb = [persist.tile([128, S], BF16, name=f"oT{r}") for r in range(NRT)]
        V_all = persist.tile([128, NS, HG, HD + 1], BF16)

        # ---- input DMAs: wq then x chunk 0 first so warmup + QKV(0) start early
        nc.sync.dma_start(wq_sb[:], wq_d[:])
        for dt in range(NK):
            nc.sync.dma_start(xT[dt][:, 0:CH], x_d[dt * 128:(dt + 1) * 128, 0:CH])
        nc.sync.dma_start(wk_sb[:], wk_d[:])
        nc.sync.dma_start(wv_sb[:], wv_d[:])
        nc.sync.dma_start(wp_sb[:], wp_d[:])
        for j in range(2):
            nc.sync.dma_start(mk_sb[:, j, :], mk_d[:])
        if has_bqk:
            nc.sync.dma_start(bq_sb[:], bq_d[:])
            nc.sync.dma_start(bk_sb[:], bk_d[:])
        if has_bv:
            nc.sync.dma_start(bv_sb[:], bv_d[:])
        if has_bp:
            nc.sync.dma_start(bp_sb[:], bp_d[:])
        for ch in range(1, NCH):
            for dt in range(NK):
                nc.sync.dma_start(
                    xT[dt][:, ch * CH:(ch + 1) * CH],
                    x_d[dt * 128:(dt + 1) * 128, ch * CH:(ch + 1) * CH],
                )

        nc.vector.memset(on1[:], 1.0)
        nc.vector.memset(V_all[:, :, :, HD:HD + 1], 1.0)

        # ---- PE warmup: keep the array busy through the HAM window while x lands
        for i in range(12):
            wps = ps_ms.tile([128, CH], F32, tag="mm", name=f"warm{i}")
            nc.tensor.matmul(wps[:], wq_sb[:, 0:128], wq_sb[:, 0:CH], start=True, stop=True)

        # ---- QKV + c_proj group emitters (each rotates one misc PSUM bank)
        def emit_qkt_group(dst, w_sb, b_sb, rt, ch):
            ps = ps_ms.tile([128, CH], F32, tag="mm", name=f"qk{rt}_{ch}")
            for kt in range(NK):
                nc.tensor.matmul(
                    ps[:],
                    w_sb[:, kt * DG + rt * 128: kt * DG + (rt + 1) * 128],
                    xT[kt][:, ch * CH:(ch + 1) * CH],
                    start=(kt == 0), stop=(kt == NK - 1),
                )
            if has_bqk:
                nc.scalar.activation(
                    dst[:, ch * CH:(ch + 1) * CH], ps[:], IDENT,
                    bias=b_sb[:, rt:rt + 1],
                )
            else:
                nc.vector.tensor_copy(dst[:, ch * CH:(ch + 1) * CH], ps[:])

        def emit_v_group(st):
            ps = ps_ms.tile([128, HG, HD], F32, tag="mm", name=f"v{st}")
            for kt in range(NK):
                nc.tensor.matmul(
                    ps[:, :, :],
                    xT[kt][:, st * 128:(st + 1) * 128],
                    wv_sb[:, kt * DG:(kt + 1) * DG],
                    start=(kt == 0), stop=(kt == NK - 1),
                )
            nc.vector.tensor_copy(V_all[:, st, :, 0:HD], ps[:, :, :])

        partials = [
            dram.tile([CHS[c], D], BF16, tag=f"partial{c}", name=f"partial{c}")
            for c in range(NCA)
        ]

        def emit_cproj_group(ci, stl, n):
            tok = T0S[ci] + stl * 128
            ps = ps_ms.tile([128, CH], F32, tag="mm", name=f"po{ci}_{stl}_{n}")
            for k2 in range(NRT):
                nc.tensor.matmul(
                    ps[:],
                    OTsb[k2][:, tok:tok + 128],
                    wp_sb[:, k2 * D + n * CH: k2 * D + (n + 1) * CH],
                    start=(k2 == 0), stop=(k2 == NRT - 1),
                )
            ob = obp.tile([128, CH], BF16, tag="ob", name=f"ob{ci}_{stl}_{n}")
            if has_bp:
                nc.vector.tensor_add(ob[:], ps[:], bp_sb[:, n * CH:(n + 1) * CH])
            else:
                nc.vector.tensor_copy(ob[:], ps[:])
            nc.sync.dma_start(
                partials[ci][stl * 128:(stl + 1) * 128, n * CH:(n + 1) * CH], ob[:]
            )

        def emit_rs(ci, half):
            # half in (0, 1) scatters 256 token rows; half=None the whole 512
            if half is None:
                rows, r0, o = 128, 0, T0S[ci] // 4
            else:
                rows, r0, o = 64, 256 * half, T0S[ci] // 4 + 64 * half
            rs_c = dram.tile([rows, D], BF16, tag=f"rs{ci}_{half}", name=f"rs_{ci}_{half}")
            nc.gpsimd.collective_compute(
                "ReduceScatter",
                mybir.AluOpType.add,
                replica_groups=[[0, 1, 2, 3], [4, 5, 6, 7]],
                ins=[partials[ci][r0:r0 + 4 * rows, :].opt()],
                outs=[rs_c.opt()],
            )
            # out DMA rides the gpsimd queue: it depends on the RS this queue
            # already blocks on, so it can never stall compute dispatch (the
            # scheduler would hoist a scalar/sync-queue DMA ahead of the exps)
            nc.gpsimd.dma_start(out_d[o:o + rows, :], rs_c[:])

        # ---- attention emitters (scores and OT split so OT trails one kt
        # behind: the PE queue always has the next scores before an exp-gated
        # OT, keeping the array streaming while ScalarE works)
        def attn_sc(ci, p, kt):
            t0k = T0S[ci] // 128
            W = CHS[ci]
            d = kt - t0k  # >=0 on the causal diagonal strip
            sc = ps_sc.tile([128, 2, CH], F32, tag="sc", name=f"sc{ci}_{p}_{kt}")
            for j in range(2):
                off = 64 * j
                nc.tensor.matmul(
                    sc[:, j, 0:W],
                    KT[p][off:off + 64, kt * 128:(kt + 1) * 128],
                    QT[p][off:off + 64, T0S[ci]:T0S[ci] + W],
                    start=True, stop=True,
                )
            st = stp.tile([128, 2, CH], BF16, tag="st", name=f"st{ci}_{p}_{kt}")
            a = max(d, 0) * 128
            nc.scalar.activation(st[:, :, a:W], sc[:, :, a:W], EXP, scale=0.125)
            if d >= 0:
                nc.vector.tensor_mul(
                    st[:, :, d * 128:(d + 1) * 128],
                    st[:, :, d * 128:(d + 1) * 128],
                    mk_sb[:, :, :],
                )
            return st

        def attn_ot(ci, p, kt, nkt, ot_pair, st):
            t0k = T0S[ci] // 128
            W = CHS[ci]
            a = max(kt - t0k, 0) * 128
            for j in range(2):
                hl = 2 * p + j
                nc.tensor.matmul(
                    ot_pair[j][:, a:W],
                    V_all[:, kt, hl, :],
                    st[:, j, a:W],
                    start=(kt == 0), stop=(kt == nkt - 1),
                )

        def pair_norm_pre(ci, p, ot_pair):
            """DVE-only reciprocal chain; returns the deferred PE+mul closure."""
            W = CHS[ci]
            den = rdp.tile([1, 2, CH], F32, tag="den", name=f"den{ci}_{p}")
            nc.vector.tensor_copy(den[:, 0, 0:W], ot_pair[0][64:65, 0:W])
            nc.vector.tensor_copy(den[:, 1, 0:W], ot_pair[1][64:65, 0:W])
            rdf = rdp.tile([1, 2, CH], F32, tag="rdf", name=f"rdf{ci}_{p}")
            nc.vector.reciprocal_approx_fast(rdf[:, :, 0:W], den[:, :, 0:W])
            if dbg:
                nc.sync.dma_start(dbg_den[:, 2 * ci + p, :, :], den[:])
            rdb = rdp.tile([1, 2, CH], BF16, tag="rdb", name=f"rdb{ci}_{p}")
            nc.vector.tensor_copy(rdb[:, :, 0:W], rdf[:, :, 0:W])

            def post():
                for j in range(2):
                    # rank-1 broadcast of the reciprocal row to 64 partitions;
                    # one PSUM tile per matmul (HW quirk, see module docstring)
                    rbp = ps_ms.tile([64, CH], F32, tag="mm", name=f"rb{ci}_{p}_{j}")
                    nc.tensor.matmul(rbp[:, 0:W], on1[:], rdb[:, j, 0:W], start=True, stop=True)
                    rbs = rdp.tile([64, CH], BF16, tag=f"rbs{j}", name=f"rbs{ci}_{p}_{j}")
                    nc.vector.tensor_copy(rbs[:, 0:W], rbp[:, 0:W])
                    dst = OTsb[p][64 * j:64 * j + 64, T0S[ci]:T0S[ci] + W]
                    nc.vector.tensor_mul(dst, ot_pair[j][0:64, 0:W], rbs[:, 0:W])
                    if has_bv:
                        nc.vector.tensor_scalar_add(
                            dst, dst, bv_sb[64 * j:64 * j + 64, p:p + 1]
                        )
            return post

        def qt_groups(ch):
            return [
                (lambda rt=rt, ch=ch: emit_qkt_group(QT[rt], wq_sb, bq_sb, rt, ch))
                for rt in range(NRT)
            ]

        def ktv_groups(ch):
            gs = [
                (lambda rt=rt, ch=ch: emit_qkt_group(KT[rt], wk_sb, bk_sb, rt, ch))
                for rt in range(NRT)
            ]
            for st4 in range(4):
                gs.append(lambda st=4 * ch + st4: emit_v_group(st))
            return gs

        def cproj_part(ci, half):
            stls = range(4) if half is None else (2 * half, 2 * half + 1)
            gs = []
            for stl in stls:
                for n in range(NRT):
                    gs.append(lambda ci=ci, stl=stl, n=n: emit_cproj_group(ci, stl, n))
            gs.append(lambda ci=ci, half=half: emit_rs(ci, half))
            return gs

        # ---- schedule. Chunks run smallest-first (0,2,3,1) so the first RS
        # enters the collective stream right after the startup barrier; each
        # chunk's c_proj runs immediately (fully or half) so its RS triggers
        # early, with remaining halves used as PE filler inside later chunks'
        # exp-bound attention. KT/V of QKV-chunk c are only read by attention
        # units with kt >= 4c, so they slide into the chunk itself under a
        # unit deadline.
        pending_post = [None]

        def attn_chunk(ci, fill_seqs, tail_fill=()):
            W = CHS[ci]
            nkt = (T0S[ci] + W) // 128
            n_units = 2 * nkt
            seqs = [(list(gs), span if span is not None else n_units)
                    for gs, span in fill_seqs]
            ui = 0
            # flatten: assign each filler an emission deadline unit
            flat = []
            u0 = 0
            for gs, span in seqs:
                for k, g in enumerate(gs):
                    flat.append((g, u0 + ((k + 1) * span) // len(gs)))
                u0 += span
            fi = 0
            for p in range(2):
                ot_pair = [
                    ps_ot.tile([65, CH], F32, tag="ot", name=f"ot{ci}_{p}_{j}")
                    for j in range(2)
                ]
                st_prev = None
                for kt in range(nkt):
                    st = attn_sc(ci, p, kt)
                    if kt == 0 and pending_post[0] is not None:
                        pending_post[0]()
                        pending_post[0] = None
                    if kt > 0:
                        attn_ot(ci, p, kt - 1, nkt, ot_pair, st_prev)
                    st_prev = st
                    ui += 1
                    while fi < len(flat) and flat[fi][1] <= ui:
                        flat[fi][0]()
                        fi += 1
                attn_ot(ci, p, nkt - 1, nkt, ot_pair, st_prev)
                pending_post[0] = pair_norm_pre(ci, p, ot_pair)
                # give the PE real work while the DVE reciprocal chain runs,
                # so the deferred rank-1 broadcast MMs don't stall it
                for _ in range(2):
                    if fi < len(flat):
                        flat[fi][0]()
                        fi += 1
            while fi < len(flat):
                flat[fi][0]()
                fi += 1
            # independent work to cover the final pair's DVE reciprocal chain,
            # then flush the norm so immediate c_proj can read OTsb
            for g in tail_fill:
                g()
            pending_post[0]()
            pending_post[0] = None

        def block(gs):
            for g in gs:
                g()

        ktv3 = ktv_groups(3)
        cp21 = cproj_part(2, 1)   # 4 MM groups + RS(2b) trigger
        cp31 = cproj_part(3, 1)   # 4 MM groups + RS(3b) trigger
        block(qt_groups(0) + ktv_groups(0) + qt_groups(1))
        attn_chunk(0, [(ktv_groups(1), None)], tail_fill=qt_groups(2))
        block(cproj_part(0, None))                         # RS(0) full, early
        attn_chunk(2, [(ktv_groups(2), 7), (qt_groups(3), None)],
                   tail_fill=ktv3[:1])
        block(cproj_part(2, 0))                            # RS(2) first half
        attn_chunk(3, [(ktv3[1:], 11), (cp21[:3], None)],
                   tail_fill=cp21[3:])
        block(cproj_part(3, 0))                            # RS(3) first half
        attn_chunk(1, [(cp31[:2], None)], tail_fill=cp31[2:])
        block(cproj_part(1, None))                         # RS(1) full = tail

        if dbg:
            for rt in range(NRT):
                nc.sync.dma_start(dbg_qt[:, rt, :], QT[rt][:])
                nc.sync.dma_start(dbg_kt[:, rt, :], KT[rt][:])
                nc.sync.dma_start(dbg_ot[:, rt, :], OTsb[rt][:])
            nc.sync.dma_start(dbg_v[:], V_all[:])
            for ci in range(NCA):
                nc.sync.dma_start(dbg_par[T0S[ci]:T0S[ci] + CHS[ci], :], partials[ci][:, :])

    nc.compile()
    return nc


_prog_cache = {}


def _get_prog(has_bv, has_bp, has_bqk):
    key = (has_bv, has_bp, has_bqk)
    if key not in _prog_cache:
        _prog_cache[key] = _build(has_bv, has_bp, has_bqk)
    return _prog_cache[key]


def _prepare(x, w_attn, b_attn, w_proj, b_proj):
    x = np.asarray(x, dtype=np.float32)
    w_attn = np.asarray(w_attn, dtype=np.float32)
    b_attn = np.asarray(b_attn, dtype=np.float32)
    w_proj = np.asarray(w_proj, dtype=np.float32)
    b_proj = np.asarray(b_proj, dtype=np.float32)

    has_bv = bool(np.any(b_attn[2 * D:]))
    has_bp = bool(np.any(b_proj))
    has_bqk = bool(np.any(b_attn[:2 * D]))
    nc = _get_prog(has_bv, has_bp, has_bqk)

    ii = np.arange(128)[:, None]
    jj = np.arange(128)[None, :]
    mask = (jj >= ii).astype(np.float32).astype(ml_dtypes.bfloat16)

    def tile_cols(w, c0, width):
        t = np.empty((128, NK * width), np.float32)
        for kt in range(NK):
            t[:, kt * width:(kt + 1) * width] = w[kt * 128:(kt + 1) * 128, c0:c0 + width]
        return t.astype(ml_dtypes.bfloat16)

    xb = [np.ascontiguousarray(x[b].T).astype(ml_dtypes.bfloat16) for b in range(B)]

    per_group = []
    for g in range(HG):
        q0 = g * DG
        k0 = D + g * DG
        v0 = 2 * D + g * DG
        wp_t = np.empty((128, NRT * D), np.float32)
        for rt in range(NRT):
            wp_t[:, rt * D:(rt + 1) * D] = w_proj[g * DG + rt * 128: g * DG + (rt + 1) * 128, :]
        bt = {}
        for nm, c0 in (("bq", q0), ("bk", k0), ("bv", v0)):
            t = np.empty((128, NRT), np.float32)
            for rt in range(NRT):
                t[:, rt] = b_attn[c0 + rt * 128: c0 + (rt + 1) * 128]
            bt[nm] = t
        if g == 0:
            bp_tile = np.broadcast_to(b_proj, (128, D)).astype(np.float32)
        else:
            bp_tile = np.zeros((128, D), dtype=np.float32)
        per_group.append({
            "wq": tile_cols(w_attn, q0, DG),
            "wk": tile_cols(w_attn, k0, DG),
            "wv": tile_cols(w_attn, v0, DG),
            "wp": wp_t.astype(ml_dtypes.bfloat16),
            "bq": np.ascontiguousarray(bt["bq"]),
            "bk": np.ascontiguousarray(bt["bk"]),
            "bv": np.ascontiguousarray(bt["bv"]),
            "bp": np.ascontiguousarray(bp_tile),
            "mk": mask,
        })

    in_maps = []
    for c in range(N_CORES):
        b, g = divmod(c, 4)
        m = dict(per_group[g])
        m["x"] = xb[b]
        in_maps.append(m)
    return nc, in_maps


def _assemble(results):
    # chunks 0 and 1 reduce-scatter whole (rank gets 128 rows); 2 and 3 in two
    # 256-token halves (rank gets 64 rows per half)
    out = np.empty((B, S, D), dtype=np.float32)
    for c in range(N_CORES):
        b, g = divmod(c, 4)
        o = np.asarray(results[c]["out"], dtype=np.float32)
        for ci in (0, 1):
            r0 = T0S[ci] // 4
            tok = T0S[ci] + g * 128
            out[b, tok:tok + 128, :] = o[r0:r0 + 128, :]
        for ci in (2, 3):
            for half in range(2):
                r0 = T0S[ci] // 4 + 64 * half
                tok = T0S[ci] + 256 * half + g * 64
                out[b, tok:tok + 64, :] = o[r0:r0 + 64, :]
    return out


def kernel(x, w_attn, b_attn, w_proj, b_proj):
    nc, in_maps = _prepare(x, w_attn, b_attn, w_proj, b_proj)
    res = run_bass_kernel_spmd(nc, in_maps, list(range(N_CORES)))
    return _assemble(res.results)
